# revision 14
# baseline (speedup 1.0000x reference)
"""Trainium2 Bass kernel for GQA attention (B=2,T=2048,D=2048,Hq=16,Hkv=4,Dh=128)
with RMSNorm + YaRN RoPE on q/k, causal softmax, out projection.

Sharding (tensor-parallel over heads, uniform SPMD across 8 cores):
  core c -> kv group g = c//2, query heads {g + 8*(c%2), g + 8*(c%2) + 4}.
  Each core computes the fused qkv projection for its 2 query heads + its kv
  group (kv work duplicated across the pair of cores sharing a group),
  RMSNorm+RoPE, full causal attention for its 2 heads, then a per-batch
  AllToAll redistributes the attention output from head-sharded to
  token-sharded so every core runs the out-projection against full Wo for
  its 1/8 token slice. Host concatenates token slices. Compute in bf16
  (f32 PSUM accumulation).
"""

import math

import numpy as np
import ml_dtypes

import concourse.bass as bass
import concourse.tile as tile
from concourse import bacc, mybir
from concourse.bass_utils import run_bass_kernel_spmd
from concourse.masks import make_identity

# ---- problem constants --------------------------------------------------
B = 2
T = 2048
D_MODEL = 2048
D_HEAD = 128
N_Q, N_KV = 16, 4
ROPE_BASE = 10000.0
YARN_SCALE = 4.0
ORIG_MAX_LEN = 4096
BETA_FAST, BETA_SLOW = 32.0, 1.0
RMS_EPS = 1.1920929e-07
MSCALE = 0.1 * math.log(YARN_SCALE) + 1.0
ATTN_SCALE = 1.0 / (MSCALE * math.sqrt(D_HEAD))

N_CORES = 8
KT = D_MODEL // 128          # 16 contraction tiles
QC = 4                       # query blocks per attention chunk
BF16 = mybir.dt.bfloat16
F32 = mybir.dt.float32
NEG_BIG = -1e30
Alu = mybir.AluOpType
Act = mybir.ActivationFunctionType


def _core_heads(c):
    g = c // 2
    ha = g + 8 * (c % 2)
    return g, (ha, ha + 4)


# o-proj: head id (Wo row block) owning A2A-out slot (r, hl); same all cores
AD_OF = [h for r in range(N_CORES) for h in _core_heads(r)[1]]


# ---- bass graph ---------------------------------------------------------

def build_nc(tb_count=T // 128, dbg=False):
    """One SPMD graph shared by all 8 cores. tb_count (multiple of 8) < 16
    gives a reduced-T variant for simulation."""
    TB = tb_count
    assert TB % N_CORES == 0 and TB % QC == 0
    BPS = TB // N_CORES            # token blocks per core after A2A
    t_tokens = TB * 128
    nc = bacc.Bacc("TRN2", target_bir_lowering=False, debug=False,
                   num_devices=N_CORES)
    dbg_t = {}
    if dbg:
        dbg_t["a2ain"] = nc.dram_tensor(
            "dbg_a2ain", [B, N_CORES, BPS, 2, 128, 128], BF16,
            kind="ExternalOutput")

    xT = nc.dram_tensor("xT", [B, D_MODEL, t_tokens], BF16, kind="ExternalInput")
    wc = nc.dram_tensor("wc", [KT, 128, 512], BF16, kind="ExternalInput")
    wo = nc.dram_tensor("wo", [KT, 128, D_MODEL], BF16, kind="ExternalInput")
    cosq = nc.dram_tensor("cosq", [t_tokens, 128], F32, kind="ExternalInput")
    sinq = nc.dram_tensor("sinq", [t_tokens, 128], F32, kind="ExternalInput")
    cosk = nc.dram_tensor("cosk", [t_tokens, 128], F32, kind="ExternalInput")
    sink = nc.dram_tensor("sink", [t_tokens, 128], F32, kind="ExternalInput")
    maskt = nc.dram_tensor("maskt", [128, 128], F32, kind="ExternalInput")
    out = nc.dram_tensor("out", [B, BPS, 128, D_MODEL], F32,
                         kind="ExternalOutput")

    with tile.TileContext(nc) as tc:
        with (
            tc.tile_pool(name="const", bufs=1) as constp,
            tc.tile_pool(name="xin", bufs=3) as xin,
            tc.tile_pool(name="wop", bufs=2) as wop,
            tc.tile_pool(name="persist", bufs=1) as persist,
            tc.tile_pool(name="work", bufs=6) as work,
            tc.tile_pool(name="outp", bufs=3) as outp,
            tc.tile_pool(name="ps512", bufs=3, space="PSUM") as ps512,
            tc.tile_pool(name="ps_att", bufs=4, space="PSUM") as ps_att,
            tc.tile_pool(name="ps_tr", bufs=1, space="PSUM") as ps_tr,
            tc.tile_pool(name="dram", bufs=1, space="DRAM") as dram,
        ):
            # ---- constants ------------------------------------------------
            wc_sb = constp.tile([128, KT, 512], BF16)
            nc.sync.dma_start(out=wc_sb, in_=wc.ap().rearrange("k p n -> p k n"))
            cs = {}
            for name, t in (("cosq", cosq), ("sinq", sinq),
                            ("cosk", cosk), ("sink", sink)):
                s = constp.tile([128, TB, 128], F32, tag=name, name=name)
                nc.gpsimd.dma_start(
                    out=s, in_=t.ap().rearrange("(tb p) d -> p tb d", p=128))
                cs[name] = s
            mask_sb = constp.tile([128, 128], F32, tag="mask", name="mask")
            nc.sync.dma_start(out=mask_sb, in_=maskt.ap())
            ident = constp.tile([128, 128], BF16, tag="ident", name="ident")
            make_identity(nc, ident)
            eps_sb = constp.tile([128, 1], F32, tag="eps", name="eps")
            nc.vector.memset(eps_sb, RMS_EPS)

            # ---- A2A bounce buffers (per batch x head) -------------------
            a2a_in, a2a_out = {}, {}
            for b in range(B):
                for hl in range(2):
                    a2a_in[(b, hl)] = dram.tile(
                        [N_CORES, BPS, 128, 128], BF16,
                        tag=f"a2ain{b}_{hl}", name=f"a2ain{b}_{hl}")
                    a2a_out[(b, hl)] = dram.tile(
                        [N_CORES, BPS, 128, 128], BF16,
                        tag=f"a2aout{b}_{hl}", name=f"a2aout{b}_{hl}")

            qTbig, kTt, vA = {}, {}, {}
            for b in range(B):
                for hl in range(2):
                    qTbig[(b, hl)] = persist.tile(
                        [128, TB * 128], BF16, tag=f"qTb{b}_{hl}",
                        name=f"qTb{b}_{hl}")

            def rope_half(src, rv, cos_ap, sin_ap, dst, tr_ps):
                """src: (128 tok,128 d) SBUF f32; rv (128,1) rsqrt; writes
                roped+normed transpose into dst (128 d, 128 tok) bf16."""
                t1 = work.tile([128, 128], F32, tag="t1", name="t1")
                nc.vector.scalar_tensor_tensor(
                    out=t1, in0=src, scalar=rv, in1=cos_ap,
                    op0=Alu.mult, op1=Alu.mult)
                t2 = work.tile([128, 128], F32, tag="t2", name="t2")
                nc.vector.scalar_tensor_tensor(
                    out=t2[:, 0:64], in0=src[:, 64:128], scalar=rv,
                    in1=sin_ap[:, 0:64], op0=Alu.mult, op1=Alu.mult)
                nc.vector.scalar_tensor_tensor(
                    out=t2[:, 64:128], in0=src[:, 0:64], scalar=rv,
                    in1=sin_ap[:, 64:128], op0=Alu.mult, op1=Alu.mult)
                qn = work.tile([128, 128], BF16, tag="qn", name="qn")
                nc.vector.tensor_add(qn, t1, t2)
                nc.tensor.transpose(tr_ps, qn, ident)
                nc.vector.tensor_copy(dst, tr_ps)

            # ---- phase 1: fused qkv projection + rope/norm ---------------
            for b in range(B):
                for tb in range(TB):
                    xt = xin.tile([128, KT, 128], BF16, tag="xt", name="xt")
                    nc.sync.dma_start(
                        out=xt,
                        in_=xT.ap()[b].rearrange("(k p) t -> p k t", p=128)
                            [:, :, tb * 128:(tb + 1) * 128])
                    ps = ps512.tile([128, 512], F32, tag="p512", name="p512")
                    for kt in range(KT):
                        nc.tensor.matmul(ps, xt[:, kt, :], wc_sb[:, kt, :],
                                         start=(kt == 0), stop=(kt == KT - 1))
                    # copy q0|q1|k to sbuf, batched rsqrt over the 3 slices
                    qs = work.tile([128, 384], F32, tag="qs", name="qs",
                                   bufs=3)
                    nc.vector.tensor_copy(qs, ps[:, 0:384])
                    sq = work.tile([128, 384], F32, tag="sq", name="sq",
                                   bufs=3)
                    ssq3 = work.tile([128, 3], F32, tag="ssq", name="ssq")
                    for i in range(3):
                        nc.vector.scalar_tensor_tensor(
                            out=sq[:, i * 128:(i + 1) * 128],
                            in0=qs[:, i * 128:(i + 1) * 128], scalar=1.0,
                            in1=qs[:, i * 128:(i + 1) * 128],
                            op0=Alu.mult, op1=Alu.mult,
                            accum_out=ssq3[:, i:i + 1])
                    std3 = work.tile([128, 3], F32, tag="std", name="std")
                    nc.scalar.activation(std3, ssq3, Act.Sqrt,
                                         bias=eps_sb, scale=1.0 / 128.0)
                    rv3 = work.tile([128, 3], F32, tag="rv", name="rv")
                    nc.vector.reciprocal(rv3, std3)
                    for hl in range(2):
                        tr = ps_tr.tile([128, 128], BF16, tag="tr", name="tr")
                        rope_half(qs[:, hl * 128:(hl + 1) * 128],
                                  rv3[:, hl:hl + 1],
                                  cs["cosq"][:, tb, :], cs["sinq"][:, tb, :],
                                  qTbig[(b, hl)][:, tb * 128:(tb + 1) * 128],
                                  tr)
                    tr = ps_tr.tile([128, 128], BF16, tag="tr", name="tr")
                    dst = persist.tile([128, 128], BF16, tag=f"kT_{b}_{tb}",
                                       name=f"kT_{b}_{tb}")
                    kTt[(b, tb)] = dst
                    rope_half(qs[:, 256:384], rv3[:, 2:3],
                              cs["cosk"][:, tb, :], cs["sink"][:, tb, :],
                              dst, tr)
                    va = persist.tile([128, 129], BF16, tag=f"vA_{b}_{tb}",
                                      name=f"vA_{b}_{tb}")
                    vA[(b, tb)] = va
                    nc.vector.tensor_copy(va[:, 0:128], ps[:, 384:512])
                    nc.vector.memset(va[:, 128:129], 1.0)

            # ---- phase 2+3+4: attention, per-(b,hl) A2A, out-proj --------
            def attn_phase(b, hl):
                qTb = qTbig[(b, hl)]
                for j in range(TB // QC):
                    q0 = j * QC
                    pas = [ps_att.tile([128, 129], F32, tag="pa",
                                       name="pa") for _ in range(QC)]

                    def score_row(kb):
                        diag = kb >= q0
                        w = (q0 + QC - kb) if diag else QC
                        cols = slice((kb if diag else q0) * 128,
                                     (q0 + QC) * 128)
                        ss = ps512.tile([128, 512], F32, tag="p512",
                                        name="p512")
                        nc.tensor.matmul(ss[:, 0:w * 128], kTt[(b, kb)],
                                         qTb[:, cols],
                                         start=True, stop=True)
                        ptw = work.tile([128, 512], BF16, tag="ptw",
                                        name="ptw")
                        if diag:
                            sm = work.tile([128, 128], F32, tag="sm",
                                           name="sm")
                            nc.vector.scalar_tensor_tensor(
                                out=sm, in0=ss[:, 0:128],
                                scalar=ATTN_SCALE, in1=mask_sb,
                                op0=Alu.mult, op1=Alu.add)
                            nc.scalar.activation(ptw[:, 0:128], sm, Act.Exp)
                            if w > 1:
                                nc.scalar.activation(
                                    ptw[:, 128:w * 128], ss[:, 128:w * 128],
                                    Act.Exp, scale=ATTN_SCALE)
                        else:
                            nc.scalar.activation(ptw, ss, Act.Exp,
                                                 scale=ATTN_SCALE)
                        return ptw

                    def pv_row(kb, ptw):
                        diag = kb >= q0
                        for qq in range(kb - q0 if diag else 0, QC):
                            off = (qq - (kb - q0)) if diag else qq
                            nc.tensor.matmul(
                                pas[qq], ptw[:, off * 128:(off + 1) * 128],
                                vA[(b, kb)],
                                start=(kb == 0), stop=(q0 + qq == kb))

                    def tail(qq):
                        qb = q0 + qq
                        pa = pas[qq]
                        rv = work.tile([128, 1], F32, tag="rsum",
                                       name="rsum")
                        nc.vector.reciprocal(rv, pa[:, 128:129])
                        an = work.tile([128, 128], BF16, tag="attn_n",
                                       name="attn_n")
                        nc.vector.tensor_scalar_mul(an, pa[:, 0:128], rv)
                        tr = ps_tr.tile([128, 128], BF16, tag="tr",
                                        name="tr")
                        nc.tensor.transpose(tr, an, ident)
                        at = work.tile([128, 128], BF16, tag="attnT",
                                       name="attnT")
                        nc.vector.tensor_copy(at, tr)
                        nc.sync.dma_start(
                            out=a2a_in[(b, hl)][qb // BPS, qb % BPS],
                            in_=at)
                        if dbg:
                            nc.sync.dma_start(
                                out=dbg_t["a2ain"].ap()[b, qb // BPS,
                                                        qb % BPS, hl],
                                in_=at)

                    # software-pipelined: score row kb+1 issues before pv of
                    # row kb so the PE never waits on ACT's exp; tails
                    # interleave one row late for the same reason.
                    rows = list(range(q0 + QC))
                    prev = None
                    for i, kb in enumerate(rows):
                        cur = score_row(kb)
                        if prev is not None:
                            pv_row(rows[i - 1], prev)
                        if kb > q0:
                            tail(kb - q0 - 1)
                        prev = cur
                    pv_row(rows[-1], prev)
                    tail(QC - 1)

            def a2a_fire(b, hl):
                nc.gpsimd.collective_compute(
                    "AllToAll", Alu.bypass,
                    replica_groups=[list(range(N_CORES))],
                    ins=[a2a_in[(b, hl)].opt()],
                    outs=[a2a_out[(b, hl)].opt()])

            def oproj_phase(b):
                gs = {}
                for blk in range(BPS):
                    g = persist.tile([128, 16, 128], BF16,
                                     tag=f"aG_{b}_{blk}", name=f"aG_{b}_{blk}")
                    gs[blk] = g
                    gr = g.rearrange("p (r hl) t -> p r hl t", hl=2)
                    for hl in range(2):
                        nc.scalar.dma_start(
                            out=gr[:, :, hl, :],
                            in_=a2a_out[(b, hl)][:, blk, :, :]
                                .rearrange("r p t -> p r t"))
                for chunk in range(4):
                    ws = wop.tile([128, KT, 512], BF16, tag="ws", name="ws")
                    nc.scalar.dma_start(
                        out=ws,
                        in_=wo.ap().rearrange("k p n -> p k n")
                            [:, :, chunk * 512:(chunk + 1) * 512])
                    for blk in range(BPS):
                        po = ps512.tile([128, 512], F32, tag="p512",
                                        name="p512")
                        for idx in range(16):
                            nc.tensor.matmul(po, gs[blk][:, idx, :],
                                             ws[:, AD_OF[idx], :],
                                             start=(idx == 0), stop=(idx == 15))
                        os_ = outp.tile([128, 512], F32, tag="os", name="os")
                        nc.vector.tensor_copy(os_, po)
                        nc.scalar.dma_start(
                            out=out.ap()[b, blk, :,
                                         chunk * 512:(chunk + 1) * 512],
                            in_=os_)

            # emission order hides each A2A behind later attention work
            attn_phase(0, 0)
            a2a_fire(0, 0)
            attn_phase(0, 1)
            a2a_fire(0, 1)
            attn_phase(1, 0)
            a2a_fire(1, 0)
            oproj_phase(0)
            attn_phase(1, 1)
            a2a_fire(1, 1)
            oproj_phase(1)
    nc.compile()
    return nc


# ---- host side ----------------------------------------------------------

def _yarn_tables(t_tokens):
    inv = 1.0 / ROPE_BASE ** (np.arange(0, D_HEAD, 2, dtype=np.float32) / D_HEAD)
    wavelengths = 2.0 * math.pi / inv
    low_wl = ORIG_MAX_LEN / BETA_SLOW
    high_wl = ORIG_MAX_LEN / BETA_FAST
    gamma = np.clip((low_wl - wavelengths) / (low_wl - high_wl), 0.0, 1.0)
    inv_freq = (gamma * inv + (1.0 - gamma) * inv / YARN_SCALE).astype(np.float32)
    t = np.arange(t_tokens, dtype=np.float32)
    freqs = np.outer(t, inv_freq)                      # (T, 64)
    emb = np.concatenate([freqs, freqs], axis=-1)      # (T, 128)
    return np.cos(emb).astype(np.float32), np.sin(emb).astype(np.float32)


def _host_prep(x, Wq, Wkv, Wo, q_norm_w, k_norm_w, tb_count=T // 128):
    t_tokens = tb_count * 128
    bf = ml_dtypes.bfloat16
    xT = np.ascontiguousarray(
        x[:, :t_tokens, :].transpose(0, 2, 1)).astype(bf)   # (B, D, T)
    cos, sin = _yarn_tables(t_tokens)
    sinF = sin.copy()
    sinF[:, :64] *= -1.0
    # rms weight applies to x before rope; the sin term reads the *rotated*
    # input, so its weight index is the input position (rolled by 64).
    wq_roll = np.concatenate([q_norm_w[64:], q_norm_w[:64]])
    wk_roll = np.concatenate([k_norm_w[64:], k_norm_w[:64]])
    cosq = np.ascontiguousarray(cos * q_norm_w[None, :])
    sinq = np.ascontiguousarray(sinF * wq_roll[None, :])
    cosk = np.ascontiguousarray(cos * k_norm_w[None, :])
    sink = np.ascontiguousarray(sinF * wk_roll[None, :])
    maskt = np.where(np.arange(128)[:, None] <= np.arange(128)[None, :],
                     0.0, NEG_BIG).astype(np.float32)       # [k, q]
    Wk, Wv = Wkv[:, :N_KV * D_HEAD], Wkv[:, N_KV * D_HEAD:]
    wo_t = np.ascontiguousarray(Wo.astype(bf).reshape(KT, 128, D_MODEL))
    in_maps = []
    for c in range(N_CORES):
        g, (ha, hb) = _core_heads(c)
        wcols = np.concatenate([
            Wq[:, ha * 128:(ha + 1) * 128], Wq[:, hb * 128:(hb + 1) * 128],
            Wk[:, g * 128:(g + 1) * 128], Wv[:, g * 128:(g + 1) * 128],
        ], axis=1).astype(bf)                               # (D, 512)
        in_maps.append({
            "xT": xT, "wc": np.ascontiguousarray(wcols.reshape(KT, 128, 512)),
            "wo": wo_t,
            "cosq": cosq, "sinq": sinq, "cosk": cosk, "sink": sink,
            "maskt": maskt,
        })
    return in_maps


def _assemble(results, tb_count=T // 128):
    BPS = tb_count // N_CORES
    t_tokens = tb_count * 128
    out = np.empty((B, t_tokens, D_MODEL), dtype=np.float32)
    for c in range(N_CORES):
        oc = results[c]["out"]              # (B, BPS, 128, D)
        for b in range(B):
            for blk in range(BPS):
                t0 = (c * BPS + blk) * 128
                out[b, t0:t0 + 128, :] = oc[b, blk]
    return out


_NC_CACHE = {}


def kernel(x, Wq, Wkv, Wo, q_norm_w, k_norm_w):
    x = np.asarray(x, dtype=np.float32)
    Wq = np.asarray(Wq, dtype=np.float32)
    Wkv = np.asarray(Wkv, dtype=np.float32)
    Wo = np.asarray(Wo, dtype=np.float32)
    q_norm_w = np.asarray(q_norm_w, dtype=np.float32)
    k_norm_w = np.asarray(k_norm_w, dtype=np.float32)

    if "nc" not in _NC_CACHE:
        _NC_CACHE["nc"] = build_nc()
    nc = _NC_CACHE["nc"]
    in_maps = _host_prep(x, Wq, Wkv, Wo, q_norm_w, k_norm_w)
    res = run_bass_kernel_spmd(nc, in_maps, core_ids=list(range(N_CORES)))
    return _assemble(res.results)


if __name__ == "__main__":
    rng = np.random.default_rng(0)
    x = rng.standard_normal((B, T, D_MODEL), dtype=np.float32)
    Wq = rng.standard_normal((D_MODEL, N_Q * D_HEAD), dtype=np.float32) * 0.02
    Wkv = rng.standard_normal((D_MODEL, 2 * N_KV * D_HEAD), dtype=np.float32) * 0.02
    Wo = rng.standard_normal((N_Q * D_HEAD, D_MODEL), dtype=np.float32) * 0.02
    w1 = np.ones(D_HEAD, dtype=np.float32)
    o = kernel(x, Wq, Wkv, Wo, w1, w1)
    print(o.shape, o.dtype, float(np.abs(o).mean()))


# revision 15
# speedup vs baseline: 1.1059x; 1.1059x over previous
"""Trainium2 Bass kernel for GQA attention (B=2,T=2048,D=2048,Hq=16,Hkv=4,Dh=128)
with RMSNorm + YaRN RoPE on q/k, causal softmax, out projection.

Sharding (tensor-parallel over heads, uniform SPMD across 8 cores):
  core c -> kv group g = c//2, query heads {g + 8*(c%2), g + 8*(c%2) + 4}.
  Each core computes the fused qkv projection for its 2 query heads + its kv
  group (kv work duplicated across the pair of cores sharing a group),
  RMSNorm+RoPE, full causal attention for its 2 heads, then a per-batch
  AllToAll redistributes the attention output from head-sharded to
  token-sharded so every core runs the out-projection against full Wo for
  its 1/8 token slice. Host concatenates token slices. Compute in bf16
  (f32 PSUM accumulation).
"""

import math

import numpy as np
import ml_dtypes

import concourse.bass as bass
import concourse.tile as tile
from concourse import bacc, mybir
from concourse.bass_utils import run_bass_kernel_spmd
from concourse.masks import make_identity

# ---- problem constants --------------------------------------------------
B = 2
T = 2048
D_MODEL = 2048
D_HEAD = 128
N_Q, N_KV = 16, 4
ROPE_BASE = 10000.0
YARN_SCALE = 4.0
ORIG_MAX_LEN = 4096
BETA_FAST, BETA_SLOW = 32.0, 1.0
RMS_EPS = 1.1920929e-07
MSCALE = 0.1 * math.log(YARN_SCALE) + 1.0
ATTN_SCALE = 1.0 / (MSCALE * math.sqrt(D_HEAD))

N_CORES = 8
KT = D_MODEL // 128          # 16 contraction tiles
QC = 4                       # query blocks per attention chunk
BF16 = mybir.dt.bfloat16
F32 = mybir.dt.float32
NEG_BIG = -1e30
Alu = mybir.AluOpType
Act = mybir.ActivationFunctionType


def _core_heads(c):
    g = c // 2
    ha = g + 8 * (c % 2)
    return g, (ha, ha + 4)


# o-proj: head id (Wo row block) owning A2A-out slot (r, hl); same all cores
AD_OF = [h for r in range(N_CORES) for h in _core_heads(r)[1]]


# ---- bass graph ---------------------------------------------------------

def build_nc(tb_count=T // 128, dbg=False):
    """One SPMD graph shared by all 8 cores. tb_count (multiple of 8) < 16
    gives a reduced-T variant for simulation."""
    TB = tb_count
    assert TB % N_CORES == 0 and TB % QC == 0
    BPS = TB // N_CORES            # token blocks per core after A2A
    t_tokens = TB * 128
    nc = bacc.Bacc("TRN2", target_bir_lowering=False, debug=False,
                   num_devices=N_CORES)
    dbg_t = {}
    if dbg:
        dbg_t["a2ain"] = nc.dram_tensor(
            "dbg_a2ain", [B, N_CORES, BPS, 2, 128, 128], BF16,
            kind="ExternalOutput")

    xT = nc.dram_tensor("xT", [B, D_MODEL, t_tokens], BF16, kind="ExternalInput")
    wc = nc.dram_tensor("wc", [KT, 128, 512], BF16, kind="ExternalInput")
    wo = nc.dram_tensor("wo", [KT, 128, D_MODEL], BF16, kind="ExternalInput")
    cosq = nc.dram_tensor("cosq", [t_tokens, 128], F32, kind="ExternalInput")
    sinq = nc.dram_tensor("sinq", [t_tokens, 128], F32, kind="ExternalInput")
    cosk = nc.dram_tensor("cosk", [t_tokens, 128], F32, kind="ExternalInput")
    sink = nc.dram_tensor("sink", [t_tokens, 128], F32, kind="ExternalInput")
    maskt = nc.dram_tensor("maskt", [128, 128], F32, kind="ExternalInput")
    out = nc.dram_tensor("out", [B, BPS, 128, D_MODEL], F32,
                         kind="ExternalOutput")

    with tile.TileContext(nc) as tc:
        with (
            tc.tile_pool(name="const", bufs=1) as constp,
            tc.tile_pool(name="xin", bufs=3) as xin,
            tc.tile_pool(name="wop", bufs=2) as wop,
            tc.tile_pool(name="persist", bufs=1) as persist,
            tc.tile_pool(name="work", bufs=6) as work,
            tc.tile_pool(name="outp", bufs=3) as outp,
            tc.tile_pool(name="ps512", bufs=3, space="PSUM") as ps512,
            tc.tile_pool(name="ps_att", bufs=4, space="PSUM") as ps_att,
            tc.tile_pool(name="ps_tr", bufs=1, space="PSUM") as ps_tr,
            tc.tile_pool(name="dram", bufs=1, space="DRAM") as dram,
        ):
            # ---- constants ------------------------------------------------
            # prefetch the first x column-blocks ahead of the big consts so
            # the first projection matmul starts as early as possible
            xt_pre = {}
            for tb in range(2):
                xt = xin.tile([128, KT, 128], BF16, tag="xt", name="xt")
                nc.sync.dma_start(
                    out=xt,
                    in_=xT.ap()[0].rearrange("(k p) t -> p k t", p=128)
                        [:, :, tb * 128:(tb + 1) * 128])
                xt_pre[(0, tb)] = xt
            wc_sb = constp.tile([128, KT, 512], BF16)
            nc.sync.dma_start(out=wc_sb, in_=wc.ap().rearrange("k p n -> p k n"))
            cs = {}
            for name, t in (("cosq", cosq), ("sinq", sinq),
                            ("cosk", cosk), ("sink", sink)):
                s = constp.tile([128, TB, 128], F32, tag=name, name=name)
                nc.gpsimd.dma_start(
                    out=s, in_=t.ap().rearrange("(tb p) d -> p tb d", p=128))
                cs[name] = s
            mask_sb = constp.tile([128, 128], F32, tag="mask", name="mask")
            nc.sync.dma_start(out=mask_sb, in_=maskt.ap())
            ident = constp.tile([128, 128], BF16, tag="ident", name="ident")
            make_identity(nc, ident)
            eps_sb = constp.tile([128, 1], F32, tag="eps", name="eps")
            nc.vector.memset(eps_sb, RMS_EPS)

            # ---- A2A bounce buffers (per batch x head) -------------------
            a2a_in, a2a_out = {}, {}
            for b in range(B):
                for hl in range(2):
                    a2a_in[(b, hl)] = dram.tile(
                        [N_CORES, BPS, 128, 128], BF16,
                        tag=f"a2ain{b}_{hl}", name=f"a2ain{b}_{hl}")
                    a2a_out[(b, hl)] = dram.tile(
                        [N_CORES, BPS, 128, 128], BF16,
                        tag=f"a2aout{b}_{hl}", name=f"a2aout{b}_{hl}")

            qTbig, kTt, vA = {}, {}, {}
            for b in range(B):
                for hl in range(2):
                    qTbig[(b, hl)] = persist.tile(
                        [128, TB * 128], BF16, tag=f"qTb{b}_{hl}",
                        name=f"qTb{b}_{hl}")

            def rope_half(src, rv, cos_ap, sin_ap, dst, tr_ps):
                """src: (128 tok,128 d) SBUF f32; rv (128,1) rsqrt; writes
                roped+normed transpose into dst (128 d, 128 tok) bf16."""
                t1 = work.tile([128, 128], F32, tag="t1", name="t1")
                nc.vector.scalar_tensor_tensor(
                    out=t1, in0=src, scalar=rv, in1=cos_ap,
                    op0=Alu.mult, op1=Alu.mult)
                t2 = work.tile([128, 128], F32, tag="t2", name="t2")
                nc.vector.scalar_tensor_tensor(
                    out=t2[:, 0:64], in0=src[:, 64:128], scalar=rv,
                    in1=sin_ap[:, 0:64], op0=Alu.mult, op1=Alu.mult)
                nc.vector.scalar_tensor_tensor(
                    out=t2[:, 64:128], in0=src[:, 0:64], scalar=rv,
                    in1=sin_ap[:, 64:128], op0=Alu.mult, op1=Alu.mult)
                qn = work.tile([128, 128], BF16, tag="qn", name="qn")
                nc.vector.tensor_add(qn, t1, t2)
                nc.tensor.transpose(tr_ps, qn, ident)
                nc.vector.tensor_copy(dst, tr_ps)

            # ---- phase 1: fused qkv projection + rope/norm ---------------
            for b in range(B):
                for tb in range(TB):
                    if (b, tb) in xt_pre:
                        xt = xt_pre[(b, tb)]
                    else:
                        xt = xin.tile([128, KT, 128], BF16, tag="xt",
                                      name="xt")
                        nc.sync.dma_start(
                            out=xt,
                            in_=xT.ap()[b].rearrange("(k p) t -> p k t", p=128)
                                [:, :, tb * 128:(tb + 1) * 128])
                    ps = ps512.tile([128, 512], F32, tag="p512", name="p512")
                    for kt in range(KT):
                        nc.tensor.matmul(ps, xt[:, kt, :], wc_sb[:, kt, :],
                                         start=(kt == 0), stop=(kt == KT - 1))
                    # batched rsqrt: squares+row-sums on ACT (psum src)
                    sqscr = work.tile([128, 128], BF16, tag="sqscr",
                                      name="sqscr")
                    ssq3 = work.tile([128, 3], F32, tag="ssq", name="ssq")
                    for i in range(3):
                        nc.scalar.activation(sqscr,
                                             ps[:, i * 128:(i + 1) * 128],
                                             Act.Square,
                                             accum_out=ssq3[:, i:i + 1])
                    std3 = work.tile([128, 3], F32, tag="std", name="std")
                    nc.scalar.activation(std3, ssq3, Act.Sqrt,
                                         bias=eps_sb, scale=1.0 / 128.0)
                    rv3 = work.tile([128, 3], F32, tag="rv", name="rv")
                    nc.vector.reciprocal(rv3, std3)
                    for hl in range(2):
                        tr = ps_tr.tile([128, 128], BF16, tag="tr", name="tr")
                        rope_half(ps[:, hl * 128:(hl + 1) * 128],
                                  rv3[:, hl:hl + 1],
                                  cs["cosq"][:, tb, :], cs["sinq"][:, tb, :],
                                  qTbig[(b, hl)][:, tb * 128:(tb + 1) * 128],
                                  tr)
                    tr = ps_tr.tile([128, 128], BF16, tag="tr", name="tr")
                    dst = persist.tile([128, 128], BF16, tag=f"kT_{b}_{tb}",
                                       name=f"kT_{b}_{tb}")
                    kTt[(b, tb)] = dst
                    rope_half(ps[:, 256:384], rv3[:, 2:3],
                              cs["cosk"][:, tb, :], cs["sink"][:, tb, :],
                              dst, tr)
                    va = persist.tile([128, 129], BF16, tag=f"vA_{b}_{tb}",
                                      name=f"vA_{b}_{tb}")
                    vA[(b, tb)] = va
                    nc.vector.tensor_copy(va[:, 0:128], ps[:, 384:512])
                    nc.vector.memset(va[:, 128:129], 1.0)

            # ---- phase 2+3+4: attention, per-(b,hl) A2A, out-proj --------
            def attn_phase(b, hl):
                qTb = qTbig[(b, hl)]
                for j in range(TB // QC):
                    q0 = j * QC
                    pas = [ps_att.tile([128, 129], F32, tag="pa",
                                       name="pa") for _ in range(QC)]

                    def score_row(kb):
                        diag = kb >= q0
                        w = (q0 + QC - kb) if diag else QC
                        cols = slice((kb if diag else q0) * 128,
                                     (q0 + QC) * 128)
                        ss = ps512.tile([128, 512], F32, tag="p512",
                                        name="p512")
                        nc.tensor.matmul(ss[:, 0:w * 128], kTt[(b, kb)],
                                         qTb[:, cols],
                                         start=True, stop=True)
                        ptw = work.tile([128, 512], BF16, tag="ptw",
                                        name="ptw")
                        if diag:
                            sm = work.tile([128, 128], F32, tag="sm",
                                           name="sm")
                            nc.vector.scalar_tensor_tensor(
                                out=sm, in0=ss[:, 0:128],
                                scalar=ATTN_SCALE, in1=mask_sb,
                                op0=Alu.mult, op1=Alu.add)
                            nc.scalar.activation(ptw[:, 0:128], sm, Act.Exp)
                            if w > 1:
                                nc.scalar.activation(
                                    ptw[:, 128:w * 128], ss[:, 128:w * 128],
                                    Act.Exp, scale=ATTN_SCALE)
                        else:
                            nc.scalar.activation(ptw, ss, Act.Exp,
                                                 scale=ATTN_SCALE)
                        return ptw

                    def pv_row(kb, ptw):
                        diag = kb >= q0
                        for qq in range(kb - q0 if diag else 0, QC):
                            off = (qq - (kb - q0)) if diag else qq
                            nc.tensor.matmul(
                                pas[qq], ptw[:, off * 128:(off + 1) * 128],
                                vA[(b, kb)],
                                start=(kb == 0), stop=(q0 + qq == kb))

                    def tail(qq):
                        qb = q0 + qq
                        pa = pas[qq]
                        rv = work.tile([128, 1], F32, tag="rsum",
                                       name="rsum")
                        nc.vector.reciprocal(rv, pa[:, 128:129])
                        an = work.tile([128, 128], BF16, tag="attn_n",
                                       name="attn_n")
                        nc.vector.tensor_scalar_mul(an, pa[:, 0:128], rv)
                        tr = ps_tr.tile([128, 128], BF16, tag="tr",
                                        name="tr")
                        nc.tensor.transpose(tr, an, ident)
                        at = work.tile([128, 128], BF16, tag="attnT",
                                       name="attnT")
                        nc.vector.tensor_copy(at, tr)
                        nc.sync.dma_start(
                            out=a2a_in[(b, hl)][qb // BPS, qb % BPS],
                            in_=at)
                        if dbg:
                            nc.sync.dma_start(
                                out=dbg_t["a2ain"].ap()[b, qb // BPS,
                                                        qb % BPS, hl],
                                in_=at)

                    # software-pipelined: score row kb+1 issues before pv of
                    # row kb so the PE never waits on ACT's exp; tails
                    # interleave one row late for the same reason.
                    rows = list(range(q0 + QC))
                    prev = None
                    for i, kb in enumerate(rows):
                        cur = score_row(kb)
                        if prev is not None:
                            pv_row(rows[i - 1], prev)
                        if kb > q0:
                            tail(kb - q0 - 1)
                        prev = cur
                    pv_row(rows[-1], prev)
                    tail(QC - 1)

            def a2a_fire(b, hl):
                nc.gpsimd.collective_compute(
                    "AllToAll", Alu.bypass,
                    replica_groups=[list(range(N_CORES))],
                    ins=[a2a_in[(b, hl)].opt()],
                    outs=[a2a_out[(b, hl)].opt()])

            def oproj_phase(b):
                gs = {}
                for blk in range(BPS):
                    g = persist.tile([128, 16, 128], BF16,
                                     tag=f"aG_{b}_{blk}", name=f"aG_{b}_{blk}")
                    gs[blk] = g
                    gr = g.rearrange("p (r hl) t -> p r hl t", hl=2)
                    for hl in range(2):
                        nc.scalar.dma_start(
                            out=gr[:, :, hl, :],
                            in_=a2a_out[(b, hl)][:, blk, :, :]
                                .rearrange("r p t -> p r t"))
                for chunk in range(4):
                    ws = wop.tile([128, KT, 512], BF16, tag="ws", name="ws")
                    nc.scalar.dma_start(
                        out=ws,
                        in_=wo.ap().rearrange("k p n -> p k n")
                            [:, :, chunk * 512:(chunk + 1) * 512])
                    for blk in range(BPS):
                        po = ps512.tile([128, 512], F32, tag="p512",
                                        name="p512")
                        for idx in range(16):
                            nc.tensor.matmul(po, gs[blk][:, idx, :],
                                             ws[:, AD_OF[idx], :],
                                             start=(idx == 0), stop=(idx == 15))
                        os_ = outp.tile([128, 512], F32, tag="os", name="os")
                        nc.vector.tensor_copy(os_, po)
                        nc.scalar.dma_start(
                            out=out.ap()[b, blk, :,
                                         chunk * 512:(chunk + 1) * 512],
                            in_=os_)

            # emission order hides each A2A behind later attention work
            attn_phase(0, 0)
            a2a_fire(0, 0)
            attn_phase(0, 1)
            a2a_fire(0, 1)
            attn_phase(1, 0)
            a2a_fire(1, 0)
            oproj_phase(0)
            attn_phase(1, 1)
            a2a_fire(1, 1)
            oproj_phase(1)
    nc.compile()
    return nc


# ---- host side ----------------------------------------------------------

def _yarn_tables(t_tokens):
    inv = 1.0 / ROPE_BASE ** (np.arange(0, D_HEAD, 2, dtype=np.float32) / D_HEAD)
    wavelengths = 2.0 * math.pi / inv
    low_wl = ORIG_MAX_LEN / BETA_SLOW
    high_wl = ORIG_MAX_LEN / BETA_FAST
    gamma = np.clip((low_wl - wavelengths) / (low_wl - high_wl), 0.0, 1.0)
    inv_freq = (gamma * inv + (1.0 - gamma) * inv / YARN_SCALE).astype(np.float32)
    t = np.arange(t_tokens, dtype=np.float32)
    freqs = np.outer(t, inv_freq)                      # (T, 64)
    emb = np.concatenate([freqs, freqs], axis=-1)      # (T, 128)
    return np.cos(emb).astype(np.float32), np.sin(emb).astype(np.float32)


def _host_prep(x, Wq, Wkv, Wo, q_norm_w, k_norm_w, tb_count=T // 128):
    t_tokens = tb_count * 128
    bf = ml_dtypes.bfloat16
    xT = np.ascontiguousarray(
        x[:, :t_tokens, :].transpose(0, 2, 1)).astype(bf)   # (B, D, T)
    cos, sin = _yarn_tables(t_tokens)
    sinF = sin.copy()
    sinF[:, :64] *= -1.0
    # rms weight applies to x before rope; the sin term reads the *rotated*
    # input, so its weight index is the input position (rolled by 64).
    wq_roll = np.concatenate([q_norm_w[64:], q_norm_w[:64]])
    wk_roll = np.concatenate([k_norm_w[64:], k_norm_w[:64]])
    cosq = np.ascontiguousarray(cos * q_norm_w[None, :])
    sinq = np.ascontiguousarray(sinF * wq_roll[None, :])
    cosk = np.ascontiguousarray(cos * k_norm_w[None, :])
    sink = np.ascontiguousarray(sinF * wk_roll[None, :])
    maskt = np.where(np.arange(128)[:, None] <= np.arange(128)[None, :],
                     0.0, NEG_BIG).astype(np.float32)       # [k, q]
    Wk, Wv = Wkv[:, :N_KV * D_HEAD], Wkv[:, N_KV * D_HEAD:]
    wo_t = np.ascontiguousarray(Wo.astype(bf).reshape(KT, 128, D_MODEL))
    in_maps = []
    for c in range(N_CORES):
        g, (ha, hb) = _core_heads(c)
        wcols = np.concatenate([
            Wq[:, ha * 128:(ha + 1) * 128], Wq[:, hb * 128:(hb + 1) * 128],
            Wk[:, g * 128:(g + 1) * 128], Wv[:, g * 128:(g + 1) * 128],
        ], axis=1).astype(bf)                               # (D, 512)
        in_maps.append({
            "xT": xT, "wc": np.ascontiguousarray(wcols.reshape(KT, 128, 512)),
            "wo": wo_t,
            "cosq": cosq, "sinq": sinq, "cosk": cosk, "sink": sink,
            "maskt": maskt,
        })
    return in_maps


def _assemble(results, tb_count=T // 128):
    BPS = tb_count // N_CORES
    t_tokens = tb_count * 128
    out = np.empty((B, t_tokens, D_MODEL), dtype=np.float32)
    for c in range(N_CORES):
        oc = results[c]["out"]              # (B, BPS, 128, D)
        for b in range(B):
            for blk in range(BPS):
                t0 = (c * BPS + blk) * 128
                out[b, t0:t0 + 128, :] = oc[b, blk]
    return out


_NC_CACHE = {}


def kernel(x, Wq, Wkv, Wo, q_norm_w, k_norm_w):
    x = np.asarray(x, dtype=np.float32)
    Wq = np.asarray(Wq, dtype=np.float32)
    Wkv = np.asarray(Wkv, dtype=np.float32)
    Wo = np.asarray(Wo, dtype=np.float32)
    q_norm_w = np.asarray(q_norm_w, dtype=np.float32)
    k_norm_w = np.asarray(k_norm_w, dtype=np.float32)

    if "nc" not in _NC_CACHE:
        _NC_CACHE["nc"] = build_nc()
    nc = _NC_CACHE["nc"]
    in_maps = _host_prep(x, Wq, Wkv, Wo, q_norm_w, k_norm_w)
    res = run_bass_kernel_spmd(nc, in_maps, core_ids=list(range(N_CORES)))
    return _assemble(res.results)


if __name__ == "__main__":
    rng = np.random.default_rng(0)
    x = rng.standard_normal((B, T, D_MODEL), dtype=np.float32)
    Wq = rng.standard_normal((D_MODEL, N_Q * D_HEAD), dtype=np.float32) * 0.02
    Wkv = rng.standard_normal((D_MODEL, 2 * N_KV * D_HEAD), dtype=np.float32) * 0.02
    Wo = rng.standard_normal((N_Q * D_HEAD, D_MODEL), dtype=np.float32) * 0.02
    w1 = np.ones(D_HEAD, dtype=np.float32)
    o = kernel(x, Wq, Wkv, Wo, w1, w1)
    print(o.shape, o.dtype, float(np.abs(o).mean()))
